# revision 1
# baseline (speedup 1.0000x reference)
"""MultiHeadAttention TRN2 Bass kernel.

Problem: B=4, S=2048, D=1024, H=16, head_dim=64. Q,K,V all derived from
`query` (faithful to the torch module; `key`/`value` args unused).

Sharding: 8 cores = (batch b in 0..3) x (seq half in 0..1). Each core:
  - receives the FULL 2048-token query of its batch, rolled so its local
    1024 query rows come first (softmax/att are permutation-invariant in
    the key axis, so key order does not matter),
  - computes K,V for all 2048 tokens (duplicated across the 2 cores of a
    batch; +25% flops but zero collectives),
  - computes attention + output projection for its 1024 queries.

On-chip layout strategy (everything contracts over the partition dim):
  - queryT [D,2048] via PE transposes (fp32), stored f32r.
  - w_qkvT chunks via PE transposes; projections in fp32r (tf32-like,
    ~1.5e-4 matmul rel err), outputs rounded to bf16.
  - scoresT[k,q] = kT.T @ qT per head; K=64 contraction -> two heads
    row-tiled via tile_position (0,0)/(64,0) to fill the 128-wide array.
  - softmax WITHOUT max subtraction (scores ~N(0,1), max ~6 over 33M
    samples; exp stays < 1e3, fp32-safe). exp on ACT with scale=1/8
    fused; output P in bf16.
  - av: outT[d,q] = v.T @ P with v as stationary; sums row via a
    concurrent col-tiled ones-matmul into the same PSUM bank
    (even head: av cols 0-63 / sums col 64; odd head: av cols 64-127 /
    sums col 0).
  - normalization: DVE reciprocal of the sums row -> f32r, broadcast
    across partitions with a K=128 matmul against a constant E matrix,
    then one DVE multiply fused with the PSUM->SBUF attn copy.
  - final projection in bf16 with the bias folded in as a K=1 matmul
    accumulation (lhsT=ones row, rhs=bias row).
"""
import os
import sys

sys.path.insert(0, "/opt/trn_rl_repo")

import numpy as np
import concourse.bacc as bacc
import concourse.tile as tile
import concourse.mybir as mybir
from concourse.bass_utils import run_bass_kernel_spmd
from concourse.masks import make_identity

F32 = mybir.dt.float32
F32R = mybir.dt.float32r
BF16 = mybir.dt.bfloat16
AF = mybir.ActivationFunctionType

B, S, D = 4, 2048, 1024
H, HD = 16, 64
SLOC = 1024  # local queries per core
N_CORES = 8

_CACHE = {}


def _build(reps=None):
    if reps is None:
        reps = int(os.environ.get("KERNEL_REPS", "1"))
    nc = bacc.Bacc("TRN2", target_bir_lowering=False, debug=False,
                   num_devices=N_CORES)
    q_in = nc.dram_tensor("q_in", [S, D], F32, kind="ExternalInput")
    w_qkv = nc.dram_tensor("w_qkv", [3 * D, D], F32, kind="ExternalInput")
    w_out = nc.dram_tensor("w_out", [D, D], F32, kind="ExternalInput")
    b_out = nc.dram_tensor("b_out", [D], F32, kind="ExternalInput")
    out = nc.dram_tensor("out", [SLOC, D], F32, kind="ExternalOutput")
    debug_taps = os.environ.get("DEBUG_TAPS", "0") == "1"
    taps = {}
    if debug_taps:
        taps["qtf0"] = nc.dram_tensor("qtf0", [128, S], BF16, kind="ExternalOutput")
        taps["qt0"] = nc.dram_tensor("qt0", [128, SLOC], BF16, kind="ExternalOutput")
        taps["kt0"] = nc.dram_tensor("kt0", [128, S], BF16, kind="ExternalOutput")
        taps["vt0"] = nc.dram_tensor("vt0", [128, 8, 65], BF16, kind="ExternalOutput")
        taps["attn0"] = nc.dram_tensor("attn0", [128, SLOC], BF16, kind="ExternalOutput")

    with tile.TileContext(nc) as tc:
        with tc.tile_pool(name="persist", bufs=1) as persist:
            for _rep in range(reps):
                # ---- constants ----
                ident = persist.tile([128, 128], F32, tag="ident")
                make_identity(nc, ident[:])
                ones_f32 = persist.tile([128, 1], F32, tag="ones_f32")
                nc.any.memset(ones_f32[:], 1.0)
                ones_bf = persist.tile([128, 1], BF16, tag="ones_bf")
                nc.vector.tensor_copy(ones_bf[:], ones_f32[:])
                ones_row_bf = persist.tile([1, 128], BF16, tag="ones_row")
                nc.vector.tensor_copy(ones_row_bf[:],
                                      ones_f32[0:1, 0:1].to_broadcast((1, 128)))

                # ================= phase A: queryT + projections =================
                with (
                    tc.tile_pool(name="a_ps", bufs=4, space="PSUM") as a_ps,
                    tc.tile_pool(name="qtf", bufs=1) as qtf_pool,
                    tc.tile_pool(name="b_ps", bufs=3, space="PSUM") as b_ps,
                ):
                    qTfull = [qtf_pool.tile([128, S], BF16, tag=f"qtf{d}", name=f"qtf{d}")
                              for d in range(8)]
                    qin_ctx = tc.tile_pool(name="qin", bufs=2)
                    qin_pool = qin_ctx.__enter__()
                    for t in range(16):
                        ch = qin_pool.tile([128, D], F32, tag="in_ch", name="q_ch")
                        nc.sync.dma_start(ch[:], q_in[t * 128:(t + 1) * 128, :])
                        for d in range(8):
                            ps = a_ps.tile([128, 128], F32, tag="tp")
                            nc.tensor.transpose(ps[:], ch[:, d * 128:(d + 1) * 128],
                                                ident[:])
                            nc.vector.tensor_copy(
                                qTfull[d][:, t * 128:(t + 1) * 128], ps[:])

                    if debug_taps:
                        nc.sync.dma_start(taps["qtf0"][:, :], qTfull[0][:])

                    # ---- cast w_qkv to bf16 in DRAM scratch (v rows first) ----
                    with tc.tile_pool(name="dram", bufs=1, space="DRAM") as dram_pool:
                        w_bf = dram_pool.tile([3 * D, D], BF16, tag="w_bf")
                        wo_bf = dram_pool.tile([D, D], BF16, tag="wo_bf")
                        for r in range(24):
                            rr = (r + 16) % 24  # v rows (16..23) first
                            ch = qin_pool.tile([128, D], F32, tag="in_ch",
                                               name="w_ch")
                            nc.sync.dma_start(
                                ch[:], w_qkv[rr * 128:(rr + 1) * 128, :])
                            cb = qin_pool.tile([128, D], BF16, tag="in_bf",
                                               name="w_cb")
                            nc.any.tensor_copy(cb[:], ch[:])
                            nc.sync.dma_start(w_bf[rr * 128:(rr + 1) * 128, :],
                                              cb[:])
                        for r in range(8):
                            ch = qin_pool.tile([128, D], F32, tag="in_ch",
                                               name="wo_ch")
                            nc.sync.dma_start(ch[:],
                                              w_out[r * 128:(r + 1) * 128, :])
                            cb = qin_pool.tile([128, D], BF16, tag="in_bf",
                                               name="wo_cb")
                            nc.any.tensor_copy(cb[:], ch[:])
                            nc.sync.dma_start(wo_bf[r * 128:(r + 1) * 128, :],
                                              cb[:])
                        qin_ctx.__exit__(None, None, None)

                        # ---- V projection ----
                        # even heads: [v(64)|ones] -> av rows 0-63, sums row 64
                        # odd heads: [ones|zeros(63)|v(64)] -> sums row 0,
                        #            av rows 64-127
                        vte = [persist.tile([128, 8, 65], BF16, tag=f"ve{t}",
                                            name=f"ve{t}") for t in range(16)]
                        vto = [persist.tile([128, 8, 128], BF16, tag=f"vo{t}",
                                            name=f"vo{t}") for t in range(16)]
                        for t in range(16):
                            nc.any.memset(vto[t][:], 0.0)
                            nc.vector.tensor_copy(
                                vte[t][:, :, 64:65],
                                ones_bf[:, 0:1].to_broadcast((128, 8, 1)))
                            nc.vector.tensor_copy(
                                vto[t][:, :, 0:1],
                                ones_bf[:, 0:1].to_broadcast((128, 8, 1)))
                        with tc.tile_pool(name="wv", bufs=1) as wv_pool:
                            w_vT = [wv_pool.tile([128, D], BF16, tag=f"wv{d}",
                                                 name=f"wv{d}") for d in range(8)]
                            for d in range(8):
                                nc.sync.dma_start_transpose(
                                    w_vT[d][:],
                                    w_bf[2 * D:3 * D, d * 128:(d + 1) * 128])
                            for t in range(16):
                                for nf in range(2):
                                    ps = b_ps.tile([128, 512], F32, tag="proj")
                                    for d in range(8):
                                        nc.tensor.matmul(
                                            ps[:],
                                            qTfull[d][:, t * 128:(t + 1) * 128],
                                            w_vT[d][:, nf * 512:(nf + 1) * 512],
                                            start=(d == 0), stop=(d == 7))
                                    hp0 = 4 * nf
                                    ps3 = ps[:].rearrange(
                                        "p (j x) -> p j x", x=64)
                                    nc.vector.tensor_copy(
                                        vte[t][:, hp0:hp0 + 4, 0:64],
                                        ps3[:, 0:8:2, :])
                                    nc.vector.tensor_copy(
                                        vto[t][:, hp0:hp0 + 4, 64:128],
                                        ps3[:, 1:8:2, :])

                        # ---- Q / K projections ----
                        qT = [persist.tile([128, SLOC], BF16, tag=f"qT{i}",
                                           name=f"qT{i}") for i in range(8)]
                        kT = [persist.tile([128, S], BF16, tag=f"kT{i}",
                                           name=f"kT{i}") for i in range(8)]
                        with tc.tile_pool(name="wqk", bufs=1) as wqk_pool:
                            w_qkT = [wqk_pool.tile([128, 2 * D], BF16,
                                                   tag=f"wqk{d}", name=f"wqk{d}")
                                     for d in range(8)]
                            for d in range(8):
                                nc.sync.dma_start_transpose(
                                    w_qkT[d][:],
                                    w_bf[0:2 * D, d * 128:(d + 1) * 128])
                            for fc in range(16):
                                if fc < 8:
                                    n_qc, dst = 2, qT[fc]
                                else:
                                    n_qc, dst = 4, kT[fc - 8]
                                for qc in range(n_qc):
                                    ps = b_ps.tile([128, 512], F32, tag="proj")
                                    for d in range(8):
                                        nc.tensor.matmul(
                                            ps[:],
                                            w_qkT[d][:, fc * 128:(fc + 1) * 128],
                                            qTfull[d][:, qc * 512:(qc + 1) * 512],
                                            start=(d == 0), stop=(d == 7))
                                    nc.vector.tensor_copy(
                                        dst[:, qc * 512:(qc + 1) * 512], ps[:])

                        # ---- w_out transpose ----
                        w_outT = [persist.tile([128, D], BF16, tag=f"wo{d}",
                                               name=f"wo{d}") for d in range(8)]
                        for d in range(8):
                            nc.sync.dma_start_transpose(
                                w_outT[d][:], wo_bf[:, d * 128:(d + 1) * 128])

                # ================= phase C: attention =================
                attn = [persist.tile([128, SLOC], BF16, tag=f"attn{i}", name=f"attn{i}")
                        for i in range(8)]
                zeros_f32 = persist.tile([128, 512], F32, tag="zeros_f32")
                nc.any.memset(zeros_f32[:], 0.0)
                E = persist.tile([128, 128], F32R, tag="E")
                nc.vector.tensor_copy(E[:], zeros_f32[:, 0:128])
                nc.vector.tensor_copy(E[64:65, 0:64],
                                      ones_f32[64:65, 0:1].to_broadcast((1, 64)))
                nc.vector.tensor_copy(E[0:1, 64:128],
                                      ones_f32[0:1, 0:1].to_broadcast((1, 64)))
                R_tiles = []
                for i in range(2):
                    R = persist.tile([128, 512], F32R, tag=f"R{i}", name=f"R{i}")
                    nc.vector.tensor_copy(R[:], zeros_f32[:])
                    R_tiles.append(R)

                with (
                    tc.tile_pool(name="p_pool", bufs=4) as p_pool,
                    tc.tile_pool(name="c_sb", bufs=3) as c_sb,
                    tc.tile_pool(name="sc_ps", bufs=2, space="PSUM") as sc_ps,
                    tc.tile_pool(name="av_ps", bufs=2, space="PSUM") as av_ps,
                    tc.tile_pool(name="bc_ps", bufs=1, space="PSUM") as bc_ps,
                ):
                    for hp in range(8):
                        for qc in range(2):
                            qsl = slice(qc * 512, (qc + 1) * 512)
                            av0 = av_ps.tile([128, 512], F32, tag="av")
                            av1 = av_ps.tile([128, 512], F32, tag="av")
                            nc.vector.memset(av0[:], 0.0)
                            nc.vector.memset(av1[:], 0.0)
                            for kc in range(16):
                                ksl = slice(kc * 128, (kc + 1) * 128)
                                sc2 = sc_ps.tile([128, 1024], F32, tag="sc")
                                nc.tensor.matmul(
                                    sc2[:, 0:512], kT[hp][0:64, ksl],
                                    qT[hp][0:64, qsl],
                                    start=True, stop=True, tile_position=(0, 0))
                                nc.tensor.matmul(
                                    sc2[:, 512:1024], kT[hp][64:128, ksl],
                                    qT[hp][64:128, qsl],
                                    start=True, stop=True, tile_position=(64, 0))
                                p2 = p_pool.tile([128, 1024], BF16, tag="p")
                                nc.scalar.activation(p2[:], sc2[:], AF.Exp,
                                                     scale=0.125)
                                pA = p2[:, 0:512]
                                pB = p2[:, 512:1024]
                                # even head: [v|1] -> av rows 0-63, sums 64
                                nc.tensor.matmul(
                                    av0[0:65, :], vte[kc][:, hp, :],
                                    pA, start=False, stop=(kc == 15),
                                    tile_position=(0, 0), skip_group_check=True)
                                # odd head: [1|0*63|v] -> sums 0, av 64-127
                                nc.tensor.matmul(
                                    av1[0:128, :], vto[kc][:, hp, :],
                                    pB, start=False, stop=(kc == 15),
                                    tile_position=(0, 0), skip_group_check=True)
                            # normalization
                            R = R_tiles[(hp * 2 + qc) % 2]
                            with nc.allow_low_precision(
                                    reason="softmax reciprocal rounded to f32r"):
                                nc.vector.reciprocal(R[64:65, :], av0[64:65, :])
                                nc.vector.reciprocal(R[0:1, :], av1[0:1, :])
                            bc = bc_ps.tile([128, 512], F32, tag="bc")
                            nc.tensor.matmul(bc[:], E[:], R[:], start=True,
                                             stop=True)
                            bc_sb = c_sb.tile([128, 512], F32, tag="bcsb")
                            nc.scalar.activation(bc_sb[:], bc[:], AF.Copy)
                            nc.vector.tensor_mul(attn[hp][0:64, qsl],
                                                 av0[0:64, :], bc_sb[0:64, :])
                            nc.vector.tensor_mul(attn[hp][64:128, qsl],
                                                 av1[64:128, :], bc_sb[64:128, :])

                if debug_taps:
                    nc.sync.dma_start(taps["qt0"][:, :], qT[0][:])
                    nc.sync.dma_start(taps["kt0"][:, :], kT[0][:])
                    nc.sync.dma_start(taps["vt0"][:, :, :], vte[0][:])
                    nc.sync.dma_start(taps["attn0"][:, :], attn[0][:])

                # ================= phase D: output projection =================
                bias_bf = persist.tile([1, D], BF16, tag="bias_bf")
                with (
                    tc.tile_pool(name="d_sb", bufs=3) as d_sb,
                    tc.tile_pool(name="d_ps", bufs=2, space="PSUM") as d_ps,
                ):
                    bias_f32 = d_sb.tile([1, D], F32, tag="bias_f32")
                    nc.sync.dma_start(bias_f32[:], b_out[:].unsqueeze(0))
                    nc.vector.tensor_copy(bias_bf[:], bias_f32[:])
                    for qm in range(8):
                        for nf in range(2):
                            nsl = slice(nf * 512, (nf + 1) * 512)
                            ps = d_ps.tile([128, 512], F32, tag="fin")
                            for d in range(8):
                                nc.tensor.matmul(
                                    ps[:], attn[d][:, qm * 128:(qm + 1) * 128],
                                    w_outT[d][:, nsl],
                                    start=(d == 0), stop=False)
                            nc.tensor.matmul(ps[:], ones_row_bf[:],
                                             bias_bf[:, nsl], start=False,
                                             stop=True, skip_group_check=True)
                            osb = d_sb.tile([128, 512], F32, tag="osb")
                            nc.any.tensor_copy(osb[:], ps[:])
                            nc.sync.dma_start(out[qm * 128:(qm + 1) * 128, nsl],
                                              osb[:])

    nc.compile()
    return nc


def _get_nc():
    if "nc" not in _CACHE:
        _CACHE["nc"] = _build()
    return _CACHE["nc"]


def kernel(query, key, value, w_qkv, w_out, b_out):
    query = np.ascontiguousarray(np.asarray(query), dtype=np.float32)
    w_qkv = np.ascontiguousarray(np.asarray(w_qkv), dtype=np.float32)
    w_out = np.ascontiguousarray(np.asarray(w_out), dtype=np.float32)
    b_out = np.ascontiguousarray(np.asarray(b_out), dtype=np.float32)

    nc = _get_nc()
    in_maps = []
    for c in range(N_CORES):
        b, half = divmod(c, 2)
        qb = query[b]
        if half:
            q_roll = np.ascontiguousarray(
                np.concatenate([qb[SLOC:], qb[:SLOC]], axis=0))
        else:
            q_roll = qb
        in_maps.append({"q_in": q_roll, "w_qkv": w_qkv,
                        "w_out": w_out, "b_out": b_out})

    res = run_bass_kernel_spmd(nc, in_maps, core_ids=list(range(N_CORES)))
    out = np.empty((B, S, D), dtype=np.float32)
    for c in range(N_CORES):
        b, half = divmod(c, 2)
        out[b, half * SLOC:(half + 1) * SLOC] = res.results[c]["out"]
    return out



# revision 2
# speedup vs baseline: 2.9598x; 2.9598x over previous
"""MultiHeadAttention TRN2 Bass kernel (v3, promoted).

B=4, S=2048, D=1024, H=16, head_dim=64. Q,K,V all derived from `query`.

Sharding: 8 cores = (batch 0..3) x (seq half 0..1). Each core receives its
batch's full 2048 tokens (rolled local-first), computes K,V for all 2048
(duplicated within the pair), attention + output projection for its local
1024 queries.

v3 = baseline structure with:
  - Host pre-transposed bf16 inputs (qT/wT/woT): no on-device transposes,
    casts, or DRAM weight bounce.
  - exp split across ACT (exact) and DVE (Schraudolph int16-bitcast
    approximation, ~2% rms) to break the ACT exp bottleneck.
  - AV in the baseline orientation (stationary [v|1] tiles, 512-wide
    moving P) — full moving-size amortization of PE stationary loads.
  - Normalization via E-matrix broadcast matmul (baseline scheme).
"""
import os
import sys

sys.path.insert(0, "/opt/trn_rl_repo")

import numpy as np
import ml_dtypes
import concourse.bacc as bacc
import concourse.tile as tile
import concourse.mybir as mybir
from concourse.bass_utils import run_bass_kernel_spmd

F32 = mybir.dt.float32
F32R = mybir.dt.float32r
BF16 = mybir.dt.bfloat16
I16 = mybir.dt.int16
AF = mybir.ActivationFunctionType
ALU = mybir.AluOpType

B, S, D = 4, 2048, 1024
H, HD = 16, 64
SLOC = 1024
N_CORES = 8

SCH_A = float(184.6627 / 8.0)
SCH_B = 16250.5

_CACHE = {}


def _build(reps=None):
    if reps is None:
        reps = int(os.environ.get("KERNEL_REPS", "1"))
    nc = bacc.Bacc("TRN2", target_bir_lowering=False, debug=False,
                   num_devices=N_CORES)
    qT_in = nc.dram_tensor("qT", [D, S], BF16, kind="ExternalInput")
    wT_in = nc.dram_tensor("wT", [D, 3 * D], BF16, kind="ExternalInput")
    woT_in = nc.dram_tensor("woT", [D, D], BF16, kind="ExternalInput")
    b_in = nc.dram_tensor("b_out", [D], F32, kind="ExternalInput")
    out = nc.dram_tensor("out", [SLOC, D], F32, kind="ExternalOutput")

    class EngPick:
        def __init__(self, pattern):
            self.pattern = pattern
            self.i = 0

        def __call__(self):
            c = self.pattern[self.i % len(self.pattern)]
            self.i += 1
            return c

    exp_pick = EngPick(os.environ.get("EXP_PATTERN", "FW"))
    cp_pick = EngPick(os.environ.get("COPY_PATTERN", "AV"))

    with tile.TileContext(nc) as tc:
        with tc.tile_pool(name="persist", bufs=1) as persist:
            for _rep in range(reps):
                ones_row = persist.tile([1, 128], BF16, tag="ones_row")
                nc.any.memset(ones_row[:], 1.0)
                ones_col = persist.tile([128, 1], F32, tag="ones_col")
                nc.any.memset(ones_col[:], 1.0)
                bias_f32 = persist.tile([1, D], F32, tag="bias_f32")
                nc.sync.dma_start(bias_f32[:], b_in[:].unsqueeze(0))
                bias_bf = persist.tile([1, D], BF16, tag="bias_bf")
                nc.vector.tensor_copy(bias_bf[:], bias_f32[:])

                # E matrix for partition-broadcast of the two recip rows
                zeros_f32 = persist.tile([128, 512], F32, tag="zeros_f32")
                nc.any.memset(zeros_f32[:], 0.0)
                E = persist.tile([128, 128], F32R, tag="E")
                nc.vector.tensor_copy(E[:], zeros_f32[:, 0:128])
                nc.vector.tensor_copy(
                    E[64:65, 0:64],
                    ones_col[64:65, 0:1].to_broadcast((1, 64)))
                nc.vector.tensor_copy(
                    E[0:1, 64:128],
                    ones_col[0:1, 0:1].to_broadcast((1, 64)))
                R_tiles = [persist.tile([128, 512], F32R, tag=f"R{i}",
                                        name=f"R{i}") for i in range(2)]
                for Rt in R_tiles:
                    nc.vector.tensor_copy(Rt[:], zeros_f32[:])

                qTp = [persist.tile([128, SLOC], BF16, tag=f"qTp{i}",
                                    name=f"qTp{i}") for i in range(8)]
                kT = [persist.tile([128, S], BF16, tag=f"kT{i}",
                                   name=f"kT{i}") for i in range(8)]
                vte = [persist.tile([128, 8, 65], BF16, tag=f"vte{t}",
                                    name=f"vte{t}") for t in range(16)]
                vto = [persist.tile([128, 8, 128], BF16, tag=f"vto{t}",
                                    name=f"vto{t}") for t in range(16)]
                woT = [persist.tile([128, D], BF16, tag=f"woT{i}",
                                    name=f"woT{i}") for i in range(8)]

                for d in range(8):
                    nc.sync.dma_start(woT[d][:],
                                      woT_in[d * 128:(d + 1) * 128, :])
                for t in range(16):
                    nc.gpsimd.memset(vte[t][:, :, 64:65], 1.0)
                    nc.gpsimd.memset(vto[t][:, :, 0:1], 1.0)
                    nc.gpsimd.memset(vto[t][:, :, 1:64], 0.0)

                # ============ projections ============
                with (
                    tc.tile_pool(name="qt", bufs=1) as qt_pool,
                    tc.tile_pool(name="wt", bufs=1) as wt_pool,
                    tc.tile_pool(name="b_ps", bufs=3, space="PSUM") as b_ps,
                ):
                    qT = [qt_pool.tile([128, S], BF16, tag=f"qT{i}",
                                       name=f"qT{i}") for i in range(8)]
                    wT = [wt_pool.tile([128, 3 * D], BF16, tag=f"wT{i}",
                                       name=f"wT{i}") for i in range(8)]
                    for d in range(8):
                        nc.sync.dma_start(qT[d][:],
                                          qT_in[d * 128:(d + 1) * 128, :])
                        nc.sync.dma_start(wT[d][:],
                                          wT_in[d * 128:(d + 1) * 128, :])

                    # V projection: out [tok, n]
                    for t in range(16):
                        for nf in range(2):
                            ps = b_ps.tile([128, 512], F32, tag="proj")
                            for d in range(8):
                                nc.tensor.matmul(
                                    ps[:],
                                    qT[d][:, t * 128:(t + 1) * 128],
                                    wT[d][:, 2 * D + nf * 512:
                                          2 * D + (nf + 1) * 512],
                                    start=(d == 0), stop=(d == 7))
                            ps3 = ps[:].rearrange("p (j x) -> p j x", x=64)
                            hp0 = 4 * nf
                            for dst, src in (
                                (vte[t][:, hp0:hp0 + 4, 0:64],
                                 ps3[:, 0:8:2, :]),
                                (vto[t][:, hp0:hp0 + 4, 64:128],
                                 ps3[:, 1:8:2, :]),
                            ):
                                if cp_pick() == "A":
                                    nc.scalar.activation(dst, src, AF.Copy)
                                else:
                                    nc.vector.tensor_copy(dst, src)

                    # K projection: out [n, tok]
                    for fc in range(8):
                        for qc in range(4):
                            ps = b_ps.tile([128, 512], F32, tag="proj")
                            for d in range(8):
                                nc.tensor.matmul(
                                    ps[:],
                                    wT[d][:, D + fc * 128:D + (fc + 1) * 128],
                                    qT[d][:, qc * 512:(qc + 1) * 512],
                                    start=(d == 0), stop=(d == 7))
                            dst = kT[fc][:, qc * 512:(qc + 1) * 512]
                            if cp_pick() == "A":
                                nc.scalar.activation(dst, ps[:], AF.Copy)
                            else:
                                nc.vector.tensor_copy(dst, ps[:])

                    # Q projection (local 1024)
                    for fc in range(8):
                        for qc in range(2):
                            ps = b_ps.tile([128, 512], F32, tag="proj")
                            for d in range(8):
                                nc.tensor.matmul(
                                    ps[:],
                                    wT[d][:, fc * 128:(fc + 1) * 128],
                                    qT[d][:, qc * 512:(qc + 1) * 512],
                                    start=(d == 0), stop=(d == 7))
                            dst = qTp[fc][:, qc * 512:(qc + 1) * 512]
                            if cp_pick() == "A":
                                nc.scalar.activation(dst, ps[:], AF.Copy)
                            else:
                                nc.vector.tensor_copy(dst, ps[:])

                # ============ attention ============
                attn_ctx = tc.tile_pool(name="attnbuf", bufs=1)
                attn_pool = attn_ctx.__enter__()
                attn = [attn_pool.tile([128, SLOC], BF16, tag=f"attn{i}",
                                       name=f"attn{i}") for i in range(8)]
                with (
                    tc.tile_pool(name="p2", bufs=6) as p2_pool,
                    tc.tile_pool(name="bcs", bufs=3) as bcs_pool,
                    tc.tile_pool(name="sc_ps", bufs=2, space="PSUM") as sc_ps,
                    tc.tile_pool(name="av_ps", bufs=2, space="PSUM") as av_ps,
                    tc.tile_pool(name="bc_ps", bufs=2, space="PSUM") as bc_ps,
                ):
                    for hp in range(8):
                        for qc in range(2):
                            qsl = slice(qc * 512, (qc + 1) * 512)
                            av0 = av_ps.tile([65, 512], F32, tag="av")
                            av1 = av_ps.tile([128, 512], F32, tag="av")
                            for kc in range(16):
                                ksl = slice(kc * 128, (kc + 1) * 128)
                                sc2 = sc_ps.tile([128, 1024], F32, tag="sc")
                                nc.tensor.matmul(
                                    sc2[:, 0:512], kT[hp][0:64, ksl],
                                    qTp[hp][0:64, qsl],
                                    start=True, stop=True,
                                    tile_position=(0, 0))
                                nc.tensor.matmul(
                                    sc2[:, 512:1024], kT[hp][64:128, ksl],
                                    qTp[hp][64:128, qsl],
                                    start=True, stop=True,
                                    tile_position=(64, 0))
                                p2 = p2_pool.tile([128, 1024], BF16, tag="p")
                                mode = exp_pick()
                                if mode == "F":      # full chunk on ACT
                                    nc.scalar.activation(p2[:], sc2[:],
                                                         AF.Exp, scale=0.125)
                                elif mode == "W":    # full chunk on DVE
                                    nc.vector.tensor_scalar(
                                        p2[:].bitcast(I16), sc2[:],
                                        SCH_A, SCH_B, ALU.mult, ALU.add)
                                elif mode == "A":    # ACT even / DVE odd
                                    nc.scalar.activation(
                                        p2[:, 0:512], sc2[:, 0:512],
                                        AF.Exp, scale=0.125)
                                    nc.vector.tensor_scalar(
                                        p2[:, 512:1024].bitcast(I16),
                                        sc2[:, 512:1024],
                                        SCH_A, SCH_B, ALU.mult, ALU.add)
                                else:                # DVE even / ACT odd
                                    nc.vector.tensor_scalar(
                                        p2[:, 0:512].bitcast(I16),
                                        sc2[:, 0:512],
                                        SCH_A, SCH_B, ALU.mult, ALU.add)
                                    nc.scalar.activation(
                                        p2[:, 512:1024], sc2[:, 512:1024],
                                        AF.Exp, scale=0.125)
                                nc.tensor.matmul(
                                    av0[:], vte[kc][:, hp, :], p2[:, 0:512],
                                    start=(kc == 0), stop=(kc == 15),
                                    skip_group_check=True)
                                nc.tensor.matmul(
                                    av1[:], vto[kc][:, hp, :],
                                    p2[:, 512:1024],
                                    start=(kc == 0), stop=(kc == 15),
                                    skip_group_check=True)
                            # normalization
                            R = R_tiles[(hp * 2 + qc) % 2]
                            with nc.allow_low_precision(
                                    reason="softmax recip rounded to f32r"):
                                nc.vector.reciprocal(R[64:65, :],
                                                     av0[64:65, :])
                                nc.vector.reciprocal(R[0:1, :], av1[0:1, :])
                            bc = bc_ps.tile([128, 512], F32, tag="bc")
                            nc.tensor.matmul(bc[:], E[:], R[:], start=True,
                                             stop=True)
                            bc_rd = bcs_pool.tile([128, 512], F32,
                                                  tag="bcsb", name="bcsb")
                            nc.scalar.activation(bc_rd[:], bc[:], AF.Copy)
                            nc.vector.tensor_mul(attn[hp][0:64, qsl],
                                                 av0[0:64, :], bc_rd[0:64, :])
                            nc.vector.tensor_mul(attn[hp][64:128, qsl],
                                                 av1[64:128, :],
                                                 bc_rd[64:128, :])

                # ============ output projection ============
                with (
                    tc.tile_pool(name="osb", bufs=3) as osb_pool,
                    tc.tile_pool(name="d_ps", bufs=2, space="PSUM") as d_ps,
                ):
                    for qm in range(8):
                        for nf in range(2):
                            nsl = slice(nf * 512, (nf + 1) * 512)
                            ps = d_ps.tile([128, 512], F32, tag="fin")
                            for d in range(8):
                                nc.tensor.matmul(
                                    ps[:], attn[d][:, qm * 128:(qm + 1) * 128],
                                    woT[d][:, nsl],
                                    start=(d == 0), stop=(d == 7))
                            nc.tensor.matmul(ps[:], ones_row[:],
                                             bias_bf[:, nsl], start=False,
                                             stop=False, skip_group_check=True)
                            osb = osb_pool.tile([128, 512], F32, tag="osb")
                            if cp_pick() == "A":
                                nc.scalar.activation(osb[:], ps[:], AF.Copy)
                            else:
                                nc.vector.tensor_copy(osb[:], ps[:])
                            nc.sync.dma_start(
                                out[qm * 128:(qm + 1) * 128, nsl], osb[:])
                attn_ctx.__exit__(None, None, None)

    nc.compile()
    return nc


def _get_nc():
    if "nc" not in _CACHE:
        _CACHE["nc"] = _build()
    return _CACHE["nc"]


def host_prep(query, w_qkv, w_out, b_out):
    bf = ml_dtypes.bfloat16
    wT = np.ascontiguousarray(np.asarray(w_qkv, np.float32).T.astype(bf))
    woT = np.ascontiguousarray(np.asarray(w_out, np.float32).T.astype(bf))
    b_out = np.ascontiguousarray(np.asarray(b_out), dtype=np.float32)
    in_maps = []
    for c in range(N_CORES):
        b, half = divmod(c, 2)
        qb = np.asarray(query[b], np.float32)
        if half:
            q_roll = np.concatenate([qb[SLOC:], qb[:SLOC]], axis=0)
        else:
            q_roll = qb
        qT = np.ascontiguousarray(q_roll.T.astype(bf))
        in_maps.append({"qT": qT, "wT": wT, "woT": woT, "b_out": b_out})
    return in_maps


def kernel(query, key, value, w_qkv, w_out, b_out):
    nc = _get_nc()
    in_maps = host_prep(query, w_qkv, w_out, b_out)
    res = run_bass_kernel_spmd(nc, in_maps, core_ids=list(range(N_CORES)))
    out = np.empty((B, S, D), dtype=np.float32)
    for c in range(N_CORES):
        b, half = divmod(c, 2)
        out[b, half * SLOC:(half + 1) * SLOC] = res.results[c]["out"]
    return out
